# revision 21
# baseline (speedup 1.0000x reference)
"""CentroidDistance kernel for 8 TRN2 NeuronCores.

Math (per the reference):
    dist[n, c] = sqrt(max(|x_n|^2 + |c_c|^2 - 2 x_n . c_c, 0))   [N, C]
    out[g, c]  = mean over nodes n with graph[n] == g of dist[n, c]

Strategy: data-parallel over nodes. Each of the 8 cores takes a contiguous
slice of ~N/8 nodes (graph ids are sorted, so each slice spans a narrow,
contiguous graph range). The centroid table is replicated. Per 128-node tile:

  PE : psum = -2 * x_tile . centT           (2 fp32 matmuls, K=128 each)
  DVE: sq   = (psum + xsq_scalar) + csq_row (one scalar_tensor_tensor)
  ACT: dist = sqrt(sq)                      (batched over a group of tiles)
  PE : psum_s[32m:32m+32] += S_tile.T @ dist  (one-hot band matmul ->
                                               per-graph partial sums)

Each 512-node group of tiles maps its graphs into a 32-wide band (one-hot S
built on the host); 4 groups share one PSUM bank (partition slots {0,32,64,96})
which is copied out every 4 groups. The host scatter-adds the bands into the
full [G, C] sum table and divides by per-graph counts (bincount). No device
collectives are needed.
"""

import os
import sys
import types
from contextlib import ExitStack

import numpy as np
import ml_dtypes

import concourse.bass as bass
import concourse.tile as tile
from concourse import bacc, mybir
from concourse.bass_utils import run_bass_kernel_spmd


def _enable_ntff_tracing():
    """Best-effort: register the axon NTFF profile hook so trace=True works.

    The agent image's `antenv` lacks the `axon_hooks` module the boot looks
    for; supply an equivalent in sys.modules and register the ctypes hook.
    """
    try:
        import antenv
        if "antenv.axon_hooks" not in sys.modules:
            mod = types.ModuleType("antenv.axon_hooks")
            holder = [None]
            mod.set_axon_ntff_profile_hook = lambda h: holder.__setitem__(0, h)
            mod.get_axon_ntff_profile_hook = lambda: holder[0]
            sys.modules["antenv.axon_hooks"] = mod
            antenv.axon_hooks = mod
        from antenv.axon_hooks import (get_axon_ntff_profile_hook,
                                       set_axon_ntff_profile_hook)
        if get_axon_ntff_profile_hook() is None:
            from trn_agent_boot.trn_boot import _ntff_profile_via_ctypes
            hook = _ntff_profile_via_ctypes("/opt/axon/libaxon_pjrt.so")
            if hook is not None:
                set_axon_ntff_profile_hook(hook)
        import concourse.bass_utils as _bu
        _bu.upload_artifacts = lambda tmpdir: f"local:{tmpdir}"
        return True
    except Exception as e:  # tracing is optional; never break the kernel
        print(f"(ntff tracing unavailable: {e})")
        return False

N_CORES = 8
D = 256          # feature dim
C = 512          # number of centroids
P = 128          # partitions / nodes per tile
BAND = 32        # graph band width per node group

F32 = mybir.dt.float32
BF16 = mybir.dt.bfloat16

LAST_EXEC_NS = None


def _build_program(nt: int, group: int):
    """Build the SPMD Bass program.

    nt: number of 128-node tiles per core (after padding)
    group: tiles per 32-graph band group (4 groups per PSUM output bank)
    """
    nc = bacc.Bacc("TRN2", target_bir_lowering=False, debug=False)

    slab = 4 * group                       # tiles per output PSUM bank
    nslabs = (nt + slab - 1) // slab
    npad = nt * P

    # x is laid out per slab as [128, 2*W]: both 128-row d-chunks of the
    # slab's nodes side by side, so one DMA (one wait sem) loads a slab.
    xT = nc.dram_tensor("xT", [P, 2 * npad], F32, kind="ExternalInput").ap()
    xsq = nc.dram_tensor("xsq", [P, nt], F32, kind="ExternalInput").ap()
    centT2 = nc.dram_tensor("centT2", [D, C], F32, kind="ExternalInput").ap()
    csq = nc.dram_tensor("csq", [P, C], F32, kind="ExternalInput").ap()
    S = nc.dram_tensor("S", [P, nt * BAND], BF16, kind="ExternalInput").ap()
    out = nc.dram_tensor("out_sums", [nslabs * P, C], F32, kind="ExternalOutput").ap()

    add = mybir.AluOpType.add
    SQRT = mybir.ActivationFunctionType.Sqrt

    with tile.TileContext(nc) as tc, ExitStack() as ctx:
        const = ctx.enter_context(tc.tile_pool(name="const", bufs=1))
        xin = ctx.enter_context(tc.tile_pool(name="xin", bufs=3))
        sqp = ctx.enter_context(tc.tile_pool(name="sq", bufs=3))
        distp = ctx.enter_context(tc.tile_pool(name="dist", bufs=3))
        stagep = ctx.enter_context(tc.tile_pool(name="stage", bufs=2))
        pmm = ctx.enter_context(tc.tile_pool(name="pmm", bufs=4, space="PSUM"))
        psums = ctx.enter_context(tc.tile_pool(name="psums", bufs=2, space="PSUM"))

        # Resident constants
        cent0 = const.tile([P, C], F32, tag="cent0")
        cent1 = const.tile([P, C], F32, tag="cent1")
        csq_sb = const.tile([P, C], F32, tag="csq")
        xsq_sb = const.tile([P, nt], F32, tag="xsq")
        s_sb = const.tile([P, nt * BAND], BF16, tag="s")
        nc.sync.dma_start(out=cent0[:], in_=centT2[0:P, :])
        nc.sync.dma_start(out=cent1[:], in_=centT2[P:D, :])
        nc.sync.dma_start(out=csq_sb[:], in_=csq[:, :])
        nc.sync.dma_start(out=xsq_sb[:], in_=xsq[:, :])
        nc.sync.dma_start(out=s_sb[:], in_=S[:, :])

        for s in range(nslabs):
            t0 = s * slab
            tiles_here = min(slab, nt - t0)
            w = tiles_here * P
            xab = xin.tile([P, 2 * slab * P], F32, tag="xab")
            nc.sync.dma_start(out=xab[:, :2 * w],
                              in_=xT[:, 2 * t0 * P:2 * t0 * P + 2 * w])

            ps_s = psums.tile([P, C], F32)
            ngr = (tiles_here + group - 1) // group
            for m in range(ngr):
                gtiles = min(group, tiles_here - m * group)
                sq = sqp.tile([P, group * C], F32, tag="sq")
                for j in range(gtiles):
                    tl = m * group + j            # tile index within slab
                    t = t0 + tl                   # global tile index
                    ps = pmm.tile([P, C], F32)
                    nc.tensor.matmul(ps[:], lhsT=xab[:, tl * P:(tl + 1) * P],
                                     rhs=cent0[:], start=True, stop=False)
                    nc.tensor.matmul(ps[:], lhsT=xab[:, w + tl * P:w + (tl + 1) * P],
                                     rhs=cent1[:], start=False, stop=True)
                    nc.vector.scalar_tensor_tensor(
                        out=sq[:, j * C:(j + 1) * C], in0=ps[:],
                        scalar=xsq_sb[:, t:t + 1], in1=csq_sb[:],
                        op0=add, op1=add)
                dist = distp.tile([P, group * C], BF16, tag="dist")
                nc.scalar.activation(dist[:, :gtiles * C], sq[:, :gtiles * C], SQRT)
                for j in range(gtiles):
                    t = t0 + m * group + j
                    nc.tensor.matmul(
                        ps_s[BAND * m:BAND * (m + 1), :],
                        lhsT=s_sb[:, BAND * t:BAND * (t + 1)],
                        rhs=dist[:, j * C:(j + 1) * C],
                        start=(j == 0), stop=(j == gtiles - 1),
                        skip_group_check=True,
                        tile_position=(0, BAND * m))

            rows = BAND * ngr            # used partitions of the output bank
            stage = stagep.tile([P, C], F32, tag="stage")
            nc.scalar.copy(stage[:rows], ps_s[:rows])
            nc.sync.dma_start(out=out[s * P:s * P + rows, :], in_=stage[:rows])

    nc.compile()
    return nc


def _prep_core(xc: np.ndarray, grc: np.ndarray, nt: int, group: int):
    """Host-side prep for one core's node slice. Returns in-map arrays and
    the per-group base graph id table (or None if a band exceeds BAND)."""
    npad = nt * P
    n_real = xc.shape[0]
    ngroups = (nt + group - 1) // group

    # band bases per group of group*P nodes
    g_base = np.zeros(ngroups, dtype=np.int64)
    for gg in range(ngroups):
        lo = gg * group * P
        hi = min(lo + group * P, n_real)
        if lo >= n_real:
            g_base[gg] = 0
            continue
        gmin = int(grc[lo])
        gmax = int(grc[hi - 1])          # sorted
        if gmax - gmin >= BAND:
            return None
        g_base[gg] = gmin

    xpad = np.zeros((npad, D), dtype=np.float32)
    xpad[:n_real] = xc
    xT_full = xpad.T                                       # [D, npad]
    # per-slab layout: [128, 2*W] blocks (both d-chunks side by side)
    slab = 4 * group
    blocks = []
    for s in range((nt + slab - 1) // slab):
        a, b = s * slab * P, min((s + 1) * slab * P, npad)
        blocks.append(xT_full[0:P, a:b])
        blocks.append(xT_full[P:D, a:b])
    xT = np.ascontiguousarray(np.concatenate(blocks, axis=1))  # [P, 2*npad]
    xsq = np.einsum("nd,nd->n", xpad, xpad).astype(np.float32)
    xsq_t = np.ascontiguousarray(xsq.reshape(nt, P).T)     # [P, nt]

    # one-hot band matrix S: [P, nt*BAND] bf16
    S = np.zeros((npad, BAND), dtype=np.float32)
    node_idx = np.arange(n_real)
    gg_idx = node_idx // (group * P)
    j = grc[:n_real] - g_base[gg_idx]
    assert (j >= 0).all() and (j < BAND).all()
    S[node_idx, j] = 1.0
    S_t = np.ascontiguousarray(
        S.reshape(nt, P, BAND).transpose(1, 0, 2).reshape(P, nt * BAND)
    ).astype(ml_dtypes.bfloat16)

    return {"xT": xT, "xsq": xsq_t, "S": S_t}, g_base


def kernel(x, centroid_weight, graph, num_graphs):
    x = np.asarray(x, dtype=np.float32)
    cw = np.asarray(centroid_weight, dtype=np.float32)
    graph = np.asarray(graph).astype(np.int64)
    G = int(num_graphs)

    N = x.shape[0]
    assert x.shape[1] == D and cw.shape == (C, D)

    nc_n = (N + N_CORES - 1) // N_CORES          # nodes per core
    nt = (nc_n + P - 1) // P                     # tiles per core

    # shared centroid-derived inputs
    centT2 = np.ascontiguousarray((-2.0 * cw).T)               # [D, C]
    csq = np.einsum("cd,cd->c", cw, cw).astype(np.float32)     # [C]
    csq_t = np.ascontiguousarray(np.broadcast_to(csq[None, :], (P, C)))

    # pick the largest group size whose bands all fit in BAND graphs
    chosen = None
    for group in (4, 2, 1):
        preps = []
        ok = True
        for c in range(N_CORES):
            lo, hi = c * nc_n, min((c + 1) * nc_n, N)
            r = _prep_core(x[lo:hi], graph[lo:hi], nt, group)
            if r is None:
                ok = False
                break
            preps.append(r)
        if ok:
            chosen = (group, preps)
            break
    assert chosen is not None, "graph bands too wide even at group=1"
    group, preps = chosen

    nc = _build_program(nt, group)

    in_maps = []
    for c in range(N_CORES):
        m, _ = preps[c]
        in_maps.append({**m, "centT2": centT2, "csq": csq_t})

    trace = bool(int(os.environ.get("KERNEL_TRACE", "0")))
    if trace:
        trace = _enable_ntff_tracing()
    res = run_bass_kernel_spmd(nc, in_maps, core_ids=list(range(N_CORES)),
                               trace=trace,
                               tmpdir=os.environ.get("KERNEL_TRACE_DIR"))
    global LAST_EXEC_NS
    LAST_EXEC_NS = res.exec_time_ns
    if res.exec_time_ns is not None:
        print(f"HW exec time: {res.exec_time_ns} ns")

    # host-side gather: scatter-add band sums into the full [G, C] table
    slab = 4 * group
    nslabs = (nt + slab - 1) // slab
    ngroups = (nt + group - 1) // group
    sums = np.zeros((G, C), dtype=np.float64)
    for c in range(N_CORES):
        _, g_base = preps[c]
        st = res.results[c]["out_sums"].reshape(nslabs, 4, BAND, C)
        lo = c * nc_n
        hi = min((c + 1) * nc_n, N)
        for gg in range(ngroups):
            if gg * group * P >= hi - lo:
                break
            gb = int(g_base[gg])
            wdt = min(BAND, G - gb)
            s_, m_ = divmod(gg, 4)
            sums[gb:gb + wdt] += st[s_, m_, :wdt, :]

    counts = np.bincount(graph, minlength=G).astype(np.float64)
    out = sums / np.maximum(counts, 1.0)[:, None]
    return out.astype(np.float32)


# revision 30
# speedup vs baseline: 2.1520x; 2.1520x over previous
"""CentroidDistance kernel for 8 TRN2 NeuronCores.

Math (per the reference):
    dist[n, c] = sqrt(max(|x_n|^2 + |c_c|^2 - 2 x_n . c_c, 0))   [N, C]
    out[g, c]  = mean over nodes n with graph[n] == g of dist[n, c]

Strategy: data-parallel over nodes. Each of the 8 cores takes a contiguous
slice of ~N/8 nodes (graph ids are sorted, so each slice spans a narrow,
contiguous graph range). The centroid table is replicated. Per 128-node tile:

  PE : psum = -2 * x_tile . centT           (2 fp32 matmuls, K=128 each)
  DVE: sq   = (psum + xsq_scalar) + csq_row (one scalar_tensor_tensor)
  ACT: dist = sqrt(sq)                      (batched over a group of tiles)
  PE : psum_s[32m:32m+32] += S_tile.T @ dist  (one-hot band matmul ->
                                               per-graph partial sums)

Each 512-node group of tiles maps its graphs into a 32-wide band (one-hot S
built on the host); 4 groups share one PSUM bank (partition slots {0,32,64,96})
which is copied out every 4 groups. The host scatter-adds the bands into the
full [G, C] sum table and divides by per-graph counts (bincount). No device
collectives are needed.
"""

import os
import sys
import types
from contextlib import ExitStack

import numpy as np
import ml_dtypes

import concourse.bass as bass
import concourse.tile as tile
from concourse import bacc, mybir
from concourse.bass_utils import run_bass_kernel_spmd


def _enable_ntff_tracing():
    """Best-effort: register the axon NTFF profile hook so trace=True works.

    The agent image's `antenv` lacks the `axon_hooks` module the boot looks
    for; supply an equivalent in sys.modules and register the ctypes hook.
    """
    try:
        import antenv
        if "antenv.axon_hooks" not in sys.modules:
            mod = types.ModuleType("antenv.axon_hooks")
            holder = [None]
            mod.set_axon_ntff_profile_hook = lambda h: holder.__setitem__(0, h)
            mod.get_axon_ntff_profile_hook = lambda: holder[0]
            sys.modules["antenv.axon_hooks"] = mod
            antenv.axon_hooks = mod
        from antenv.axon_hooks import (get_axon_ntff_profile_hook,
                                       set_axon_ntff_profile_hook)
        if get_axon_ntff_profile_hook() is None:
            from trn_agent_boot.trn_boot import _ntff_profile_via_ctypes
            hook = _ntff_profile_via_ctypes("/opt/axon/libaxon_pjrt.so")
            if hook is not None:
                set_axon_ntff_profile_hook(hook)
        import concourse.bass_utils as _bu
        _bu.upload_artifacts = lambda tmpdir: f"local:{tmpdir}"
        return True
    except Exception as e:  # tracing is optional; never break the kernel
        print(f"(ntff tracing unavailable: {e})")
        return False

N_CORES = 8
D = 256          # feature dim
C = 512          # number of centroids
P = 128          # partitions / nodes per tile
BAND = 32        # graph band width per node group

F32 = mybir.dt.float32
BF16 = mybir.dt.bfloat16

LAST_EXEC_NS = None


def _build_program(nt: int, group: int):
    """Build the SPMD Bass program.

    nt: number of 128-node tiles per core (after padding)
    group: tiles per 32-graph band group (4 groups per PSUM output bank)
    """
    nc = bacc.Bacc("TRN2", target_bir_lowering=False, debug=False)

    slab = 4 * group                       # tiles per output PSUM bank
    nslabs = (nt + slab - 1) // slab
    npad = nt * P

    # x is laid out per slab as [128, 2*W]: both 128-row d-chunks of the
    # slab's nodes side by side, so one DMA (one wait sem) loads a slab.
    # bf16: fp32 matmuls take TWO PE passes (FP32 HI/LO), bf16 takes one.
    xT = nc.dram_tensor("xT", [P, 2 * npad], BF16, kind="ExternalInput").ap()
    xsq = nc.dram_tensor("xsq", [P, nt], F32, kind="ExternalInput").ap()
    centT2 = nc.dram_tensor("centT2", [D, C], BF16, kind="ExternalInput").ap()
    csq = nc.dram_tensor("csq", [P, 2 * C], F32, kind="ExternalInput").ap()
    S = nc.dram_tensor("S", [P, nt * BAND], BF16, kind="ExternalInput").ap()
    out = nc.dram_tensor("out_sums", [nslabs * P, C], F32, kind="ExternalOutput").ap()

    add = mybir.AluOpType.add
    SQRT = mybir.ActivationFunctionType.Sqrt

    with tile.TileContext(nc) as tc, ExitStack() as ctx:
        const = ctx.enter_context(tc.tile_pool(name="const", bufs=1))
        xin = ctx.enter_context(tc.tile_pool(name="xin", bufs=3))
        sqp = ctx.enter_context(tc.tile_pool(name="sq", bufs=4))
        distp = ctx.enter_context(tc.tile_pool(name="dist", bufs=6))
        stagep = ctx.enter_context(tc.tile_pool(name="stage", bufs=2))
        pmm = ctx.enter_context(tc.tile_pool(name="pmm", bufs=3, space="PSUM"))
        psums = ctx.enter_context(tc.tile_pool(name="psums", bufs=2, space="PSUM"))

        # Resident constants
        cent0 = const.tile([P, C], BF16, tag="cent0")
        cent1 = const.tile([P, C], BF16, tag="cent1")
        csq_sb = const.tile([P, 2 * C], F32, tag="csq")
        xsq_sb = const.tile([P, nt], F32, tag="xsq")
        s_sb = const.tile([P, nt * BAND], BF16, tag="s")
        nc.sync.dma_start(out=cent0[:], in_=centT2[0:P, :])
        nc.sync.dma_start(out=cent1[:], in_=centT2[P:D, :])
        nc.sync.dma_start(out=csq_sb[:], in_=csq[:, :])
        nc.sync.dma_start(out=xsq_sb[:], in_=xsq[:, :])
        nc.sync.dma_start(out=s_sb[:], in_=S[:, :])

        for s in range(nslabs):
            t0 = s * slab
            tiles_here = min(slab, nt - t0)
            w = tiles_here * P
            xab = xin.tile([P, 2 * slab * P], BF16, tag="xab")
            nc.sync.dma_start(out=xab[:, :2 * w],
                              in_=xT[:, 2 * t0 * P:2 * t0 * P + 2 * w])

            ps_s = psums.tile([P, C], F32)
            ngr = (tiles_here + group - 1) // group
            # pairs of tiles share a 2-bank PSUM tile so the DVE +csq add
            # runs once per pair (amortizes the per-op DVE overhead)
            for pr in range((tiles_here + 1) // 2):
                ptiles = min(2, tiles_here - pr * 2)
                ps = pmm.tile([P, 2 * C], F32)
                for j in range(ptiles):
                    tl = pr * 2 + j               # tile index within slab
                    nc.tensor.matmul(ps[:, j * C:(j + 1) * C],
                                     lhsT=xab[:, tl * P:(tl + 1) * P],
                                     rhs=cent0[:], start=True, stop=False)
                    nc.tensor.matmul(ps[:, j * C:(j + 1) * C],
                                     lhsT=xab[:, w + tl * P:w + (tl + 1) * P],
                                     rhs=cent1[:], start=False, stop=True)
                sq = sqp.tile([P, 2 * C], F32, tag="sq")
                nc.vector.tensor_tensor(
                    out=sq[:, :ptiles * C], in0=ps[:, :ptiles * C],
                    in1=csq_sb[:, :ptiles * C], op=add)
                for j in range(ptiles):
                    tl = pr * 2 + j
                    t = t0 + tl
                    m = tl // group
                    dist = distp.tile([P, C], BF16, tag="dist")
                    nc.scalar.activation(dist[:], sq[:, j * C:(j + 1) * C],
                                         SQRT, bias=xsq_sb[:, t:t + 1])
                    nc.tensor.matmul(
                        ps_s[BAND * m:BAND * (m + 1), :],
                        lhsT=s_sb[:, BAND * t:BAND * (t + 1)],
                        rhs=dist[:],
                        start=(tl % group == 0),
                        stop=(tl % group == group - 1 or tl == tiles_here - 1),
                        skip_group_check=True,
                        tile_position=(0, BAND * m))

            rows = BAND * ngr            # used partitions of the output bank
            stage = stagep.tile([P, C], F32, tag="stage")
            if s % 2 == 0:
                nc.scalar.copy(stage[:rows], ps_s[:rows])
            else:
                nc.vector.tensor_copy(stage[:rows], ps_s[:rows])
            nc.sync.dma_start(out=out[s * P:s * P + rows, :], in_=stage[:rows])

    nc.compile()
    return nc


def _prep_core(xc: np.ndarray, grc: np.ndarray, nt: int, group: int):
    """Host-side prep for one core's node slice. Returns in-map arrays and
    the per-group base graph id table (or None if a band exceeds BAND)."""
    npad = nt * P
    n_real = xc.shape[0]
    ngroups = (nt + group - 1) // group

    # band bases per group of group*P nodes
    g_base = np.zeros(ngroups, dtype=np.int64)
    for gg in range(ngroups):
        lo = gg * group * P
        hi = min(lo + group * P, n_real)
        if lo >= n_real:
            g_base[gg] = 0
            continue
        gmin = int(grc[lo])
        gmax = int(grc[hi - 1])          # sorted
        if gmax - gmin >= BAND:
            return None
        g_base[gg] = gmin

    xpad = np.zeros((npad, D), dtype=np.float32)
    xpad[:n_real] = xc
    xT_full = xpad.T                                       # [D, npad]
    # per-slab layout: [128, 2*W] blocks (both d-chunks side by side)
    slab = 4 * group
    blocks = []
    for s in range((nt + slab - 1) // slab):
        a, b = s * slab * P, min((s + 1) * slab * P, npad)
        blocks.append(xT_full[0:P, a:b])
        blocks.append(xT_full[P:D, a:b])
    xT = np.ascontiguousarray(
        np.concatenate(blocks, axis=1)).astype(ml_dtypes.bfloat16)
    xsq = np.einsum("nd,nd->n", xpad, xpad).astype(np.float32)
    xsq_t = np.ascontiguousarray(xsq.reshape(nt, P).T)     # [P, nt]

    # one-hot band matrix S: [P, nt*BAND] bf16
    S = np.zeros((npad, BAND), dtype=np.float32)
    node_idx = np.arange(n_real)
    gg_idx = node_idx // (group * P)
    j = grc[:n_real] - g_base[gg_idx]
    assert (j >= 0).all() and (j < BAND).all()
    S[node_idx, j] = 1.0
    S_t = np.ascontiguousarray(
        S.reshape(nt, P, BAND).transpose(1, 0, 2).reshape(P, nt * BAND)
    ).astype(ml_dtypes.bfloat16)

    return {"xT": xT, "xsq": xsq_t, "S": S_t}, g_base


def kernel(x, centroid_weight, graph, num_graphs):
    x = np.asarray(x, dtype=np.float32)
    cw = np.asarray(centroid_weight, dtype=np.float32)
    graph = np.asarray(graph).astype(np.int64)
    G = int(num_graphs)

    N = x.shape[0]
    assert x.shape[1] == D and cw.shape == (C, D)

    nc_n = (N + N_CORES - 1) // N_CORES          # nodes per core
    nt = (nc_n + P - 1) // P                     # tiles per core

    # shared centroid-derived inputs
    centT2 = np.ascontiguousarray((-2.0 * cw).T).astype(ml_dtypes.bfloat16)
    csq = np.einsum("cd,cd->c", cw, cw).astype(np.float32)     # [C]
    csq2 = np.concatenate([csq, csq])                          # tiled twice
    csq_t = np.ascontiguousarray(
        np.broadcast_to(csq2[None, :], (P, 2 * C)).astype(np.float32))

    # pick the largest group size whose bands all fit in BAND graphs
    chosen = None
    for group in (4, 2, 1):
        preps = []
        ok = True
        for c in range(N_CORES):
            lo, hi = c * nc_n, min((c + 1) * nc_n, N)
            r = _prep_core(x[lo:hi], graph[lo:hi], nt, group)
            if r is None:
                ok = False
                break
            preps.append(r)
        if ok:
            chosen = (group, preps)
            break
    assert chosen is not None, "graph bands too wide even at group=1"
    group, preps = chosen

    nc = _build_program(nt, group)

    in_maps = []
    for c in range(N_CORES):
        m, _ = preps[c]
        in_maps.append({**m, "centT2": centT2, "csq": csq_t})

    trace = bool(int(os.environ.get("KERNEL_TRACE", "0")))
    if trace:
        trace = _enable_ntff_tracing()
    res = run_bass_kernel_spmd(nc, in_maps, core_ids=list(range(N_CORES)),
                               trace=trace,
                               tmpdir=os.environ.get("KERNEL_TRACE_DIR"))
    global LAST_EXEC_NS
    LAST_EXEC_NS = res.exec_time_ns
    if res.exec_time_ns is not None:
        print(f"HW exec time: {res.exec_time_ns} ns")

    # host-side gather: scatter-add band sums into the full [G, C] table
    slab = 4 * group
    nslabs = (nt + slab - 1) // slab
    ngroups = (nt + group - 1) // group
    sums = np.zeros((G, C), dtype=np.float64)
    for c in range(N_CORES):
        _, g_base = preps[c]
        st = res.results[c]["out_sums"].reshape(nslabs, 4, BAND, C)
        lo = c * nc_n
        hi = min((c + 1) * nc_n, N)
        for gg in range(ngroups):
            if gg * group * P >= hi - lo:
                break
            gb = int(g_base[gg])
            wdt = min(BAND, G - gb)
            s_, m_ = divmod(gg, 4)
            sums[gb:gb + wdt] += st[s_, m_, :wdt, :]

    counts = np.bincount(graph, minlength=G).astype(np.float64)
    out = sums / np.maximum(counts, 1.0)[:, None]
    return out.astype(np.float32)


# revision 37
# speedup vs baseline: 2.7486x; 1.2772x over previous
"""CentroidDistance kernel for 8 TRN2 NeuronCores.

Math (per the reference):
    dist[n, c] = sqrt(max(|x_n|^2 + |c_c|^2 - 2 x_n . c_c, 0))   [N, C]
    out[g, c]  = mean over nodes n with graph[n] == g of dist[n, c]

Strategy: data-parallel over nodes. Each of the 8 cores takes a contiguous
slice of ~N/8 nodes (graph ids are sorted, so each slice spans a narrow,
contiguous graph range). The centroid table is replicated. Per 128-node tile:

  PE : psum = -2 * x_tile . centT           (2 fp32 matmuls, K=128 each)
  DVE: sq   = (psum + xsq_scalar) + csq_row (one scalar_tensor_tensor)
  ACT: dist = sqrt(sq)                      (batched over a group of tiles)
  PE : psum_s[32m:32m+32] += S_tile.T @ dist  (one-hot band matmul ->
                                               per-graph partial sums)

Each 512-node group of tiles maps its graphs into a 32-wide band (one-hot S
built on the host); 4 groups share one PSUM bank (partition slots {0,32,64,96})
which is copied out every 4 groups. The host scatter-adds the bands into the
full [G, C] sum table and divides by per-graph counts (bincount). No device
collectives are needed.
"""

import os
import sys
import types
from contextlib import ExitStack

import numpy as np
import ml_dtypes

import concourse.bass as bass
import concourse.tile as tile
from concourse import bacc, mybir
from concourse.bass_utils import run_bass_kernel_spmd


def _enable_ntff_tracing():
    """Best-effort: register the axon NTFF profile hook so trace=True works.

    The agent image's `antenv` lacks the `axon_hooks` module the boot looks
    for; supply an equivalent in sys.modules and register the ctypes hook.
    """
    try:
        import antenv
        if "antenv.axon_hooks" not in sys.modules:
            mod = types.ModuleType("antenv.axon_hooks")
            holder = [None]
            mod.set_axon_ntff_profile_hook = lambda h: holder.__setitem__(0, h)
            mod.get_axon_ntff_profile_hook = lambda: holder[0]
            sys.modules["antenv.axon_hooks"] = mod
            antenv.axon_hooks = mod
        from antenv.axon_hooks import (get_axon_ntff_profile_hook,
                                       set_axon_ntff_profile_hook)
        if get_axon_ntff_profile_hook() is None:
            from trn_agent_boot.trn_boot import _ntff_profile_via_ctypes
            hook = _ntff_profile_via_ctypes("/opt/axon/libaxon_pjrt.so")
            if hook is not None:
                set_axon_ntff_profile_hook(hook)
        import concourse.bass_utils as _bu
        _bu.upload_artifacts = lambda tmpdir: f"local:{tmpdir}"
        return True
    except Exception as e:  # tracing is optional; never break the kernel
        print(f"(ntff tracing unavailable: {e})")
        return False

def _patch_walrus_flags():
    """Flip --enable-ldw-opt to true: concourse disables it by default, but
    without it every LDWEIGHTS serializes with its MATMUL (~+75ns/matmul)."""
    import concourse.bass_utils as _bu
    if getattr(_bu.run_command, "_ldw_patched", False):
        return
    _orig = _bu.run_command

    def run_command_ldw(cmd, **kw):
        if isinstance(cmd, list):
            cmd = ["--enable-ldw-opt=true" if c == "--enable-ldw-opt=false" else c
                   for c in cmd]
        return _orig(cmd, **kw)

    run_command_ldw._ldw_patched = True
    _bu.run_command = run_command_ldw


if int(os.environ.get("KERNEL_LDW_OPT", "1")):
    _patch_walrus_flags()

N_CORES = 8
D = 256          # feat dim
C = 512          # number of centroids
P = 128          # partitions / nodes per tile
BAND = 32        # graph band width per node group

F32 = mybir.dt.float32
BF16 = mybir.dt.bfloat16

LAST_EXEC_NS = None


def _build_program(nt: int, group: int):
    """Build the SPMD Bass program.

    nt: number of 128-node tiles per core (after padding)
    group: tiles per 32-graph band group (4 groups per PSUM output bank)
    """
    nc = bacc.Bacc("TRN2", target_bir_lowering=False, debug=False)

    slab = 4 * group                       # tiles per output PSUM bank
    nslabs = (nt + slab - 1) // slab
    npad = nt * P

    # x is laid out per slab as [128, 2*W]: both 128-row d-chunks of the
    # slab's nodes side by side, so one DMA (one wait sem) loads a slab.
    # fp8e4m3 + DoubleRow: one matmul contracts all 256 d-rows (2 per PE
    # cell); the exact |x|^2 / |c|^2 terms stay fp32 so only the small
    # cross-term -2x.c carries fp8 rounding.
    FP8 = mybir.dt.float8e4
    xT = nc.dram_tensor("xT", [P, 2 * npad], FP8, kind="ExternalInput").ap()
    xsq = nc.dram_tensor("xsq", [P, nt], F32, kind="ExternalInput").ap()
    centT2 = nc.dram_tensor("centT2", [P, 2 * C], FP8, kind="ExternalInput").ap()
    csq = nc.dram_tensor("csq", [P, 2 * C], F32, kind="ExternalInput").ap()
    S = nc.dram_tensor("S", [P, nt * BAND], BF16, kind="ExternalInput").ap()
    out = nc.dram_tensor("out_sums", [nslabs * P, C], F32, kind="ExternalOutput").ap()

    add = mybir.AluOpType.add
    SQRT = mybir.ActivationFunctionType.Sqrt

    with tile.TileContext(nc) as tc, ExitStack() as ctx:
        const = ctx.enter_context(tc.tile_pool(name="const", bufs=1))
        xin = ctx.enter_context(tc.tile_pool(name="xin", bufs=3))
        sqp = ctx.enter_context(tc.tile_pool(name="sq", bufs=4))
        distp = ctx.enter_context(tc.tile_pool(name="dist", bufs=6))
        stagep = ctx.enter_context(tc.tile_pool(name="stage", bufs=2))
        pmm = ctx.enter_context(tc.tile_pool(name="pmm", bufs=3, space="PSUM"))
        psums = ctx.enter_context(tc.tile_pool(name="psums", bufs=2, space="PSUM"))

        # Resident constants
        cent = const.tile([P, 2 * C], FP8, tag="cent")
        csq_sb = const.tile([P, 2 * C], F32, tag="csq")
        xsq_sb = const.tile([P, nt], F32, tag="xsq")
        s_sb = const.tile([P, nt * BAND], BF16, tag="s")
        nc.sync.dma_start(out=cent[:], in_=centT2[:, :])
        nc.sync.dma_start(out=csq_sb[:], in_=csq[:, :])
        nc.sync.dma_start(out=xsq_sb[:], in_=xsq[:, :])
        nc.sync.dma_start(out=s_sb[:], in_=S[:, :])

        for s in range(nslabs):
            t0 = s * slab
            tiles_here = min(slab, nt - t0)
            w = tiles_here * P
            xab = xin.tile([P, 2 * slab * P], FP8, tag="xab")
            nc.sync.dma_start(out=xab[:, :2 * w],
                              in_=xT[:, 2 * t0 * P:2 * t0 * P + 2 * w])
            xab3 = xab[:, :2 * w].rearrange("p (two ww) -> p two ww", two=2)
            cent3 = cent[:].rearrange("p (two c) -> p two c", two=2)

            ps_s = psums.tile([P, C], F32)
            ngr = (tiles_here + group - 1) // group
            # pairs of tiles share a 2-bank PSUM tile so the DVE +csq add
            # runs once per pair (amortizes the per-op DVE overhead)
            for pr in range((tiles_here + 1) // 2):
                ptiles = min(2, tiles_here - pr * 2)
                ps = pmm.tile([P, 2 * C], F32)
                for j in range(ptiles):
                    tl = pr * 2 + j               # tile index within slab
                    nc.tensor.matmul(ps[:, j * C:(j + 1) * C],
                                     lhsT=xab3[:, :, tl * P:(tl + 1) * P],
                                     rhs=cent3[:, :, :],
                                     start=True, stop=True,
                                     perf_mode=mybir.MatmulPerfMode.DoubleRow)
                sq = sqp.tile([P, 2 * C], F32, tag="sq")
                nc.vector.tensor_tensor(
                    out=sq[:, :ptiles * C], in0=ps[:, :ptiles * C],
                    in1=csq_sb[:, :ptiles * C], op=add)
                for j in range(ptiles):
                    tl = pr * 2 + j
                    t = t0 + tl
                    m = tl // group
                    dist = distp.tile([P, C], BF16, tag="dist")
                    nc.scalar.activation(dist[:], sq[:, j * C:(j + 1) * C],
                                         SQRT, bias=xsq_sb[:, t:t + 1])
                    nc.tensor.matmul(
                        ps_s[BAND * m:BAND * (m + 1), :],
                        lhsT=s_sb[:, BAND * t:BAND * (t + 1)],
                        rhs=dist[:],
                        start=(tl % group == 0),
                        stop=(tl % group == group - 1 or tl == tiles_here - 1),
                        skip_group_check=True,
                        tile_position=(0, BAND * m))

            rows = BAND * ngr            # used partitions of the output bank
            stage = stagep.tile([P, C], F32, tag="stage")
            if s % 2 == 0:
                nc.scalar.copy(stage[:rows], ps_s[:rows])
            else:
                nc.vector.tensor_copy(stage[:rows], ps_s[:rows])
            nc.sync.dma_start(out=out[s * P:s * P + rows, :], in_=stage[:rows])

    nc.compile()
    return nc


def _prep_core(xc: np.ndarray, grc: np.ndarray, nt: int, group: int):
    """Host-side prep for one core's node slice. Returns in-map arrays and
    the per-group base graph id table (or None if a band exceeds BAND)."""
    npad = nt * P
    n_real = xc.shape[0]
    ngroups = (nt + group - 1) // group

    # band bases per group of group*P nodes
    g_base = np.zeros(ngroups, dtype=np.int64)
    for gg in range(ngroups):
        lo = gg * group * P
        hi = min(lo + group * P, n_real)
        if lo >= n_real:
            g_base[gg] = 0
            continue
        gmin = int(grc[lo])
        gmax = int(grc[hi - 1])          # sorted
        if gmax - gmin >= BAND:
            return None
        g_base[gg] = gmin

    xpad = np.zeros((npad, D), dtype=np.float32)
    xpad[:n_real] = xc
    xT_full = xpad.T                                       # [D, npad]
    # per-slab layout: [128, 2*W] blocks (both d-chunks side by side)
    slab = 4 * group
    blocks = []
    for s in range((nt + slab - 1) // slab):
        a, b = s * slab * P, min((s + 1) * slab * P, npad)
        blocks.append(xT_full[0:P, a:b])
        blocks.append(xT_full[P:D, a:b])
    xT = np.ascontiguousarray(
        np.concatenate(blocks, axis=1)).astype(ml_dtypes.float8_e4m3)
    xsq = np.einsum("nd,nd->n", xpad, xpad).astype(np.float32)
    xsq_t = np.ascontiguousarray(xsq.reshape(nt, P).T)     # [P, nt]

    # one-hot band matrix S: [P, nt*BAND] bf16
    S = np.zeros((npad, BAND), dtype=np.float32)
    node_idx = np.arange(n_real)
    gg_idx = node_idx // (group * P)
    j = grc[:n_real] - g_base[gg_idx]
    assert (j >= 0).all() and (j < BAND).all()
    S[node_idx, j] = 1.0
    S_t = np.ascontiguousarray(
        S.reshape(nt, P, BAND).transpose(1, 0, 2).reshape(P, nt * BAND)
    ).astype(ml_dtypes.bfloat16)

    return {"xT": xT, "xsq": xsq_t, "S": S_t}, g_base


def kernel(x, centroid_weight, graph, num_graphs):
    x = np.asarray(x, dtype=np.float32)
    cw = np.asarray(centroid_weight, dtype=np.float32)
    graph = np.asarray(graph).astype(np.int64)
    G = int(num_graphs)

    N = x.shape[0]
    assert x.shape[1] == D and cw.shape == (C, D)

    nc_n = (N + N_CORES - 1) // N_CORES          # nodes per core
    nt = (nc_n + P - 1) // P                     # tiles per core

    # shared centroid-derived inputs: [128, 2C] fp8, both d-chunks per row
    c2 = (-2.0 * cw).T                                         # [D, C]
    centT2 = np.ascontiguousarray(
        np.concatenate([c2[0:P, :], c2[P:D, :]], axis=1)
    ).astype(ml_dtypes.float8_e4m3)
    csq = np.einsum("cd,cd->c", cw, cw).astype(np.float32)     # [C]
    csq2 = np.concatenate([csq, csq])                          # tiled twice
    csq_t = np.ascontiguousarray(
        np.broadcast_to(csq2[None, :], (P, 2 * C)).astype(np.float32))

    # pick the largest group size whose bands all fit in BAND graphs
    chosen = None
    for group in (4, 2, 1):
        preps = []
        ok = True
        for c in range(N_CORES):
            lo, hi = c * nc_n, min((c + 1) * nc_n, N)
            r = _prep_core(x[lo:hi], graph[lo:hi], nt, group)
            if r is None:
                ok = False
                break
            preps.append(r)
        if ok:
            chosen = (group, preps)
            break
    assert chosen is not None, "graph bands too wide even at group=1"
    group, preps = chosen

    nc = _build_program(nt, group)

    in_maps = []
    for c in range(N_CORES):
        m, _ = preps[c]
        in_maps.append({**m, "centT2": centT2, "csq": csq_t})

    trace = bool(int(os.environ.get("KERNEL_TRACE", "0")))
    if trace:
        trace = _enable_ntff_tracing()
    res = run_bass_kernel_spmd(nc, in_maps, core_ids=list(range(N_CORES)),
                               trace=trace,
                               tmpdir=os.environ.get("KERNEL_TRACE_DIR"))
    global LAST_EXEC_NS
    LAST_EXEC_NS = res.exec_time_ns
    if res.exec_time_ns is not None:
        print(f"HW exec time: {res.exec_time_ns} ns")

    # host-side gather: scatter-add band sums into the full [G, C] table
    slab = 4 * group
    nslabs = (nt + slab - 1) // slab
    ngroups = (nt + group - 1) // group
    sums = np.zeros((G, C), dtype=np.float64)
    for c in range(N_CORES):
        _, g_base = preps[c]
        st = res.results[c]["out_sums"].reshape(nslabs, 4, BAND, C)
        lo = c * nc_n
        hi = min((c + 1) * nc_n, N)
        for gg in range(ngroups):
            if gg * group * P >= hi - lo:
                break
            gb = int(g_base[gg])
            wdt = min(BAND, G - gb)
            s_, m_ = divmod(gg, 4)
            sums[gb:gb + wdt] += st[s_, m_, :wdt, :]

    counts = np.bincount(graph, minlength=G).astype(np.float64)
    out = sums / np.maximum(counts, 1.0)[:, None]
    return out.astype(np.float32)


# revision 44
# speedup vs baseline: 2.8223x; 1.0268x over previous
"""CentroidDistance kernel for 8 TRN2 NeuronCores.

Math (per the reference):
    dist[n, c] = sqrt(max(|x_n|^2 + |c_c|^2 - 2 x_n . c_c, 0))   [N, C]
    out[g, c]  = mean over nodes n with graph[n] == g of dist[n, c]

Strategy: data-parallel over nodes. Each of the 8 cores takes a contiguous
slice of ~N/8 nodes (graph ids are sorted, so each slice spans a narrow,
contiguous graph range). The centroid table is replicated. Per 128-node tile:

  PE : psum = -2 * x_tile . centT           (2 fp32 matmuls, K=128 each)
  DVE: sq   = (psum + xsq_scalar) + csq_row (one scalar_tensor_tensor)
  ACT: dist = sqrt(sq)                      (batched over a group of tiles)
  PE : psum_s[32m:32m+32] += S_tile.T @ dist  (one-hot band matmul ->
                                               per-graph partial sums)

Each 512-node group of tiles maps its graphs into a 32-wide band (one-hot S
built on the host); 4 groups share one PSUM bank (partition slots {0,32,64,96})
which is copied out every 4 groups. The host scatter-adds the bands into the
full [G, C] sum table and divides by per-graph counts (bincount). No device
collectives are needed.
"""

import os
import sys
import types
from contextlib import ExitStack

import numpy as np
import ml_dtypes

import concourse.bass as bass
import concourse.tile as tile
from concourse import bacc, mybir
from concourse.bass_utils import run_bass_kernel_spmd


def _enable_ntff_tracing():
    """Best-effort: register the axon NTFF profile hook so trace=True works.

    The agent image's `antenv` lacks the `axon_hooks` module the boot looks
    for; supply an equivalent in sys.modules and register the ctypes hook.
    """
    try:
        import antenv
        if "antenv.axon_hooks" not in sys.modules:
            mod = types.ModuleType("antenv.axon_hooks")
            holder = [None]
            mod.set_axon_ntff_profile_hook = lambda h: holder.__setitem__(0, h)
            mod.get_axon_ntff_profile_hook = lambda: holder[0]
            sys.modules["antenv.axon_hooks"] = mod
            antenv.axon_hooks = mod
        from antenv.axon_hooks import (get_axon_ntff_profile_hook,
                                       set_axon_ntff_profile_hook)
        if get_axon_ntff_profile_hook() is None:
            from trn_agent_boot.trn_boot import _ntff_profile_via_ctypes
            hook = _ntff_profile_via_ctypes("/opt/axon/libaxon_pjrt.so")
            if hook is not None:
                set_axon_ntff_profile_hook(hook)
        import concourse.bass_utils as _bu
        _bu.upload_artifacts = lambda tmpdir: f"local:{tmpdir}"
        return True
    except Exception as e:  # tracing is optional; never break the kernel
        print(f"(ntff tracing unavailable: {e})")
        return False

def _patch_walrus_flags():
    """Flip --enable-ldw-opt to true: concourse disables it by default, but
    without it every LDWEIGHTS serializes with its MATMUL (~+75ns/matmul)."""
    import concourse.bass_utils as _bu
    if getattr(_bu.run_command, "_ldw_patched", False):
        return
    _orig = _bu.run_command

    def run_command_ldw(cmd, **kw):
        if isinstance(cmd, list):
            cmd = ["--enable-ldw-opt=true" if c == "--enable-ldw-opt=false" else c
                   for c in cmd]
        return _orig(cmd, **kw)

    run_command_ldw._ldw_patched = True
    _bu.run_command = run_command_ldw


if int(os.environ.get("KERNEL_LDW_OPT", "1")):
    _patch_walrus_flags()

N_CORES = 8
D = 256          # feat dim
C = 512          # number of centroids
P = 128          # partitions / nodes per tile
BAND = 32        # graph band width per node group

F32 = mybir.dt.float32
BF16 = mybir.dt.bfloat16

LAST_EXEC_NS = None


def _build_program(nt: int, group: int):
    """Build the SPMD Bass program.

    nt: number of 128-node tiles per core (after padding)
    group: tiles per 32-graph band group (4 groups per PSUM output bank)
    """
    nc = bacc.Bacc("TRN2", target_bir_lowering=False, debug=False)

    slab = 4 * group                       # tiles per output PSUM bank
    nslabs = (nt + slab - 1) // slab
    npad = nt * P

    # x is laid out per slab as [128, 2*W]: both 128-row d-chunks of the
    # slab's nodes side by side, so one DMA (one wait sem) loads a slab.
    # fp8e4m3 + DoubleRow: one matmul contracts all 256 d-rows (2 per PE
    # cell); the exact |x|^2 / |c|^2 terms stay fp32 so only the small
    # cross-term -2x.c carries fp8 rounding.
    FP8 = mybir.dt.float8e4
    xT = nc.dram_tensor("xT", [P, 2 * npad], FP8, kind="ExternalInput").ap()
    xsq = nc.dram_tensor("xsq", [P, nt], F32, kind="ExternalInput").ap()
    centT2 = nc.dram_tensor("centT2", [P, 2 * C], FP8, kind="ExternalInput").ap()
    csq = nc.dram_tensor("csq", [P, 2 * C], F32, kind="ExternalInput").ap()
    S = nc.dram_tensor("S", [P, nt * BAND], BF16, kind="ExternalInput").ap()
    out = nc.dram_tensor("out_sums", [nslabs * P, C], F32, kind="ExternalOutput").ap()

    add = mybir.AluOpType.add
    SQRT = mybir.ActivationFunctionType.Sqrt

    with tile.TileContext(nc) as tc, ExitStack() as ctx:
        const = ctx.enter_context(tc.tile_pool(name="const", bufs=1))
        xin = ctx.enter_context(tc.tile_pool(name="xin", bufs=3))
        sqp = ctx.enter_context(tc.tile_pool(name="sq", bufs=6))
        distp = ctx.enter_context(tc.tile_pool(name="dist", bufs=6))
        stagep = ctx.enter_context(tc.tile_pool(name="stage", bufs=2))
        pmm = ctx.enter_context(tc.tile_pool(name="pmm", bufs=3, space="PSUM"))
        psums = ctx.enter_context(tc.tile_pool(name="psums", bufs=2, space="PSUM"))

        # Resident constants
        cent = const.tile([P, 2 * C], FP8, tag="cent")
        csq_sb = const.tile([P, 2 * C], F32, tag="csq")
        xsq_sb = const.tile([P, nt], F32, tag="xsq")
        s_sb = const.tile([P, nt * BAND], BF16, tag="s")
        nc.sync.dma_start(out=cent[:], in_=centT2[:, :])
        nc.sync.dma_start(out=csq_sb[:], in_=csq[:, :])
        nc.sync.dma_start(out=xsq_sb[:], in_=xsq[:, :])
        nc.sync.dma_start(out=s_sb[:], in_=S[:, :])

        for s in range(nslabs):
            t0 = s * slab
            tiles_here = min(slab, nt - t0)
            w = tiles_here * P
            xab = xin.tile([P, 2 * slab * P], FP8, tag="xab")
            nc.sync.dma_start(out=xab[:, :2 * w],
                              in_=xT[:, 2 * t0 * P:2 * t0 * P + 2 * w])
            xab3 = xab[:, :2 * w].rearrange("p (two ww) -> p two ww", two=2)
            cent3 = cent[:].rearrange("p (two c) -> p two c", two=2)

            ps_s = psums.tile([P, C], F32)
            ngr = (tiles_here + group - 1) // group
            # pairs of tiles share a 2-bank PSUM tile so the DVE +csq add
            # runs once per pair (amortizes the per-op DVE overhead)
            for pr in range((tiles_here + 1) // 2):
                ptiles = min(2, tiles_here - pr * 2)
                ps = pmm.tile([P, 2 * C], F32)
                for j in range(ptiles):
                    tl = pr * 2 + j               # tile index within slab
                    nc.tensor.matmul(ps[:, j * C:(j + 1) * C],
                                     lhsT=xab3[:, :, tl * P:(tl + 1) * P],
                                     rhs=cent3[:, :, :],
                                     start=True, stop=True,
                                     perf_mode=mybir.MatmulPerfMode.DoubleRow)
                sq = sqp.tile([P, 2 * C], F32, tag="sq")
                dist = distp.tile([P, 2 * C], BF16, tag="dist")
                if pr % 3 == 2 and ptiles == 2:
                    # path B (~1/3 of pairs): DVE does both adds per tile,
                    # ACT runs one batched bias-free sqrt over the pair —
                    # balances ACT (bias blocks batching) against DVE
                    for j in range(ptiles):
                        t = t0 + pr * 2 + j
                        nc.vector.scalar_tensor_tensor(
                            out=sq[:, j * C:(j + 1) * C],
                            in0=ps[:, j * C:(j + 1) * C],
                            scalar=xsq_sb[:, t:t + 1],
                            in1=csq_sb[:, :C], op0=add, op1=add)
                    nc.scalar.activation(dist[:], sq[:], SQRT)
                else:
                    # path A: one batched DVE add (+csq), per-tile ACT
                    # sqrt with the per-partition |x|^2 bias
                    nc.vector.tensor_tensor(
                        out=sq[:, :ptiles * C], in0=ps[:, :ptiles * C],
                        in1=csq_sb[:, :ptiles * C], op=add)
                    for j in range(ptiles):
                        t = t0 + pr * 2 + j
                        nc.scalar.activation(dist[:, j * C:(j + 1) * C],
                                             sq[:, j * C:(j + 1) * C],
                                             SQRT, bias=xsq_sb[:, t:t + 1])
                for j in range(ptiles):
                    tl = pr * 2 + j
                    t = t0 + tl
                    m = tl // group
                    nc.tensor.matmul(
                        ps_s[BAND * m:BAND * (m + 1), :],
                        lhsT=s_sb[:, BAND * t:BAND * (t + 1)],
                        rhs=dist[:, j * C:(j + 1) * C],
                        start=(tl % group == 0),
                        stop=(tl % group == group - 1 or tl == tiles_here - 1),
                        skip_group_check=True,
                        tile_position=(0, BAND * m))
            rows = BAND * ngr            # used partitions of the output bank
            stage = stagep.tile([P, C], F32, tag="stage")
            if s % 2 == 0:
                nc.scalar.copy(stage[:rows], ps_s[:rows])
            else:
                nc.vector.tensor_copy(stage[:rows], ps_s[:rows])
            nc.sync.dma_start(out=out[s * P:s * P + rows, :], in_=stage[:rows])

    nc.compile()
    return nc


def _prep_core(xc: np.ndarray, grc: np.ndarray, nt: int, group: int):
    """Host-side prep for one core's node slice. Returns in-map arrays and
    the per-group base graph id table (or None if a band exceeds BAND)."""
    npad = nt * P
    n_real = xc.shape[0]
    ngroups = (nt + group - 1) // group

    # band bases per group of group*P nodes
    g_base = np.zeros(ngroups, dtype=np.int64)
    for gg in range(ngroups):
        lo = gg * group * P
        hi = min(lo + group * P, n_real)
        if lo >= n_real:
            g_base[gg] = 0
            continue
        gmin = int(grc[lo])
        gmax = int(grc[hi - 1])          # sorted
        if gmax - gmin >= BAND:
            return None
        g_base[gg] = gmin

    xpad = np.zeros((npad, D), dtype=np.float32)
    xpad[:n_real] = xc
    xT_full = xpad.T                                       # [D, npad]
    # per-slab layout: [128, 2*W] blocks (both d-chunks side by side)
    slab = 4 * group
    blocks = []
    for s in range((nt + slab - 1) // slab):
        a, b = s * slab * P, min((s + 1) * slab * P, npad)
        blocks.append(xT_full[0:P, a:b])
        blocks.append(xT_full[P:D, a:b])
    xT = np.ascontiguousarray(
        np.concatenate(blocks, axis=1)).astype(ml_dtypes.float8_e4m3)
    xsq = np.einsum("nd,nd->n", xpad, xpad).astype(np.float32)
    xsq_t = np.ascontiguousarray(xsq.reshape(nt, P).T)     # [P, nt]

    # one-hot band matrix S: [P, nt*BAND] bf16
    S = np.zeros((npad, BAND), dtype=np.float32)
    node_idx = np.arange(n_real)
    gg_idx = node_idx // (group * P)
    j = grc[:n_real] - g_base[gg_idx]
    assert (j >= 0).all() and (j < BAND).all()
    S[node_idx, j] = 1.0
    S_t = np.ascontiguousarray(
        S.reshape(nt, P, BAND).transpose(1, 0, 2).reshape(P, nt * BAND)
    ).astype(ml_dtypes.bfloat16)

    return {"xT": xT, "xsq": xsq_t, "S": S_t}, g_base


def kernel(x, centroid_weight, graph, num_graphs):
    x = np.asarray(x, dtype=np.float32)
    cw = np.asarray(centroid_weight, dtype=np.float32)
    graph = np.asarray(graph).astype(np.int64)
    G = int(num_graphs)

    N = x.shape[0]
    assert x.shape[1] == D and cw.shape == (C, D)

    nc_n = (N + N_CORES - 1) // N_CORES          # nodes per core
    nt = (nc_n + P - 1) // P                     # tiles per core

    # shared centroid-derived inputs: [128, 2C] fp8, both d-chunks per row
    c2 = (-2.0 * cw).T                                         # [D, C]
    centT2 = np.ascontiguousarray(
        np.concatenate([c2[0:P, :], c2[P:D, :]], axis=1)
    ).astype(ml_dtypes.float8_e4m3)
    csq = np.einsum("cd,cd->c", cw, cw).astype(np.float32)     # [C]
    csq2 = np.concatenate([csq, csq])                          # tiled twice
    csq_t = np.ascontiguousarray(
        np.broadcast_to(csq2[None, :], (P, 2 * C)).astype(np.float32))

    # pick the largest group size whose bands all fit in BAND graphs
    chosen = None
    for group in (4, 2, 1):
        preps = []
        ok = True
        for c in range(N_CORES):
            lo, hi = c * nc_n, min((c + 1) * nc_n, N)
            r = _prep_core(x[lo:hi], graph[lo:hi], nt, group)
            if r is None:
                ok = False
                break
            preps.append(r)
        if ok:
            chosen = (group, preps)
            break
    assert chosen is not None, "graph bands too wide even at group=1"
    group, preps = chosen

    nc = _build_program(nt, group)

    in_maps = []
    for c in range(N_CORES):
        m, _ = preps[c]
        in_maps.append({**m, "centT2": centT2, "csq": csq_t})

    trace = bool(int(os.environ.get("KERNEL_TRACE", "0")))
    if trace:
        trace = _enable_ntff_tracing()
    res = run_bass_kernel_spmd(nc, in_maps, core_ids=list(range(N_CORES)),
                               trace=trace,
                               tmpdir=os.environ.get("KERNEL_TRACE_DIR"))
    global LAST_EXEC_NS
    LAST_EXEC_NS = res.exec_time_ns
    if res.exec_time_ns is not None:
        print(f"HW exec time: {res.exec_time_ns} ns")

    # host-side gather: scatter-add band sums into the full [G, C] table
    slab = 4 * group
    nslabs = (nt + slab - 1) // slab
    ngroups = (nt + group - 1) // group
    sums = np.zeros((G, C), dtype=np.float64)
    for c in range(N_CORES):
        _, g_base = preps[c]
        st = res.results[c]["out_sums"].reshape(nslabs, 4, BAND, C)
        lo = c * nc_n
        hi = min((c + 1) * nc_n, N)
        for gg in range(ngroups):
            if gg * group * P >= hi - lo:
                break
            gb = int(g_base[gg])
            wdt = min(BAND, G - gb)
            s_, m_ = divmod(gg, 4)
            sums[gb:gb + wdt] += st[s_, m_, :wdt, :]

    counts = np.bincount(graph, minlength=G).astype(np.float64)
    out = sums / np.maximum(counts, 1.0)[:, None]
    return out.astype(np.float32)
